# revision 2
# baseline (speedup 1.0000x reference)
"""Multi-head causal self-attention (B=2, S=2048, E=1024, H=16) on 8 TRN2 NeuronCores.

Sharding: tensor-parallel over heads (2 heads/core, both batches). Per core:
  - QKV projection for its 2 heads (q^T/k^T transposed layout, v natural)
  - causal flash-style attention, scores computed transposed (k on partitions)
    so no probability-matrix transposes are needed; softmax denominator comes
    from a ones-column appended to V
  - PE-transpose of the attention output, 8-way AllToAll to reshard from
    head-columns to token-rows, then row-parallel output projection.
Host side only reshapes/slices inputs and concatenates the 8 disjoint row
shards of the output.
"""

import numpy as np
import ml_dtypes

P = 128
B, S, E, H, D = 2, 2048, 1024, 16, 64
NCORES = 8
EB = E // P            # 8 e-blocks
BS = B * S             # 4096 flattened rows
SBB = S // P           # 16 s-blocks per batch
SB = BS // P           # 32 s-blocks global
HPC = H // NCORES      # 2 heads per core
CH = BS // NCORES      # 512 rows owned per core

_bf16 = ml_dtypes.bfloat16
_cache = {}


def _build():
    from contextlib import ExitStack

    import concourse.tile as tile
    from concourse import bacc, mybir
    from concourse.masks import make_identity

    bf16 = mybir.dt.bfloat16
    f32 = mybir.dt.float32

    nc = bacc.Bacc("TRN2", target_bir_lowering=False, debug=False,
                   num_devices=NCORES)

    xT_d = nc.dram_tensor("xT", [EB, P, BS], bf16, kind="ExternalInput")
    wqk_d = nc.dram_tensor("wqk", [EB, P, 2 * P], bf16, kind="ExternalInput")
    wv_d = nc.dram_tensor("wv", [EB, P, P], bf16, kind="ExternalInput")
    wo_d = nc.dram_tensor("wo", [EB, P, E], bf16, kind="ExternalInput")
    bqk_d = nc.dram_tensor("bqk", [2, P], f32, kind="ExternalInput")
    bv_d = nc.dram_tensor("bv", [1, P], bf16, kind="ExternalInput")
    bo_d = nc.dram_tensor("bo", [1, E], bf16, kind="ExternalInput")
    tri_d = nc.dram_tensor("tri", [P, P], bf16, kind="ExternalInput")
    out_d = nc.dram_tensor("out", [CH, E], f32, kind="ExternalOutput")
    a2a_in = nc.dram_tensor("a2a_in", [NCORES, P, CH], bf16)
    a2a_out = nc.dram_tensor("a2a_out", [NCORES, P, CH], bf16)

    with tile.TileContext(nc) as tc, ExitStack() as ctx:
        consts = ctx.enter_context(tc.tile_pool(name="consts", bufs=1))
        work = ctx.enter_context(tc.tile_pool(name="work", bufs=1))
        epool = ctx.enter_context(tc.tile_pool(name="expst", bufs=1))
        small = ctx.enter_context(tc.tile_pool(name="small", bufs=4))
        opool = ctx.enter_context(tc.tile_pool(name="osb", bufs=2))
        pbig = ctx.enter_context(tc.tile_pool(name="pbig", bufs=4, space="PSUM"))
        psm = ctx.enter_context(tc.tile_pool(name="psm", bufs=4, space="PSUM"))

        wqk = consts.tile([P, EB, 2 * P], bf16, tag="wqk")
        wv = consts.tile([P, EB, P], bf16, tag="wv")
        wo = consts.tile([P, EB, E], bf16, tag="wo")
        bqk = consts.tile([P, 2], f32, tag="bqk")
        bv = consts.tile([1, P], bf16, tag="bv")
        bo = consts.tile([1, E], bf16, tag="bo")
        tri = consts.tile([P, P], bf16, tag="tri")
        ones1 = consts.tile([1, P], bf16, tag="ones1")
        ident = consts.tile([P, P], bf16, tag="ident")
        xT = consts.tile([P, EB, BS], bf16, tag="xT")

        for eb in range(EB):
            nc.sync.dma_start(wqk[:, eb, :], wqk_d[eb])
            nc.sync.dma_start(wv[:, eb, :], wv_d[eb])
            nc.sync.dma_start(wo[:, eb, :], wo_d[eb])
        for j in range(2):
            nc.sync.dma_start(bqk[:, j], bqk_d[j])
        nc.sync.dma_start(bv[:1, :], bv_d[:, :])
        nc.sync.dma_start(bo[:1, :], bo_d[:, :])
        nc.sync.dma_start(tri[:], tri_d[:, :])
        nc.vector.memset(ones1[:1, :], 1.0)
        make_identity(nc, ident[:])
        for eb in range(EB):
            nc.sync.dma_start(xT[:, eb, :], xT_d[eb])

        # ---- QKV projection ----
        # q^T/k^T: [dim(2 heads x 64), token] layout; q pre-scaled by 1/8 on host
        qkT = work.tile([P, 2, BS], bf16, tag="qkT")
        for db in range(2):
            for sc in range(BS // 512):
                ps = pbig.tile([P, 512], f32, tag="big")
                for eb in range(EB):
                    nc.tensor.matmul(
                        ps[:],
                        lhsT=wqk[:, eb, db * P:(db + 1) * P],
                        rhs=xT[:, eb, sc * 512:(sc + 1) * 512],
                        start=(eb == 0), stop=(eb == EB - 1),
                    )
                nc.vector.tensor_scalar_add(
                    qkT[:, db, sc * 512:(sc + 1) * 512], ps[:], bqk[:, db:db + 1])

        # v natural [token, 2 heads x 64], with a ones column at index 64 of
        # each head segment (softmax denominator trick); bias via K=1 matmul
        vsb = work.tile([P, SB, HPC, 66], bf16, tag="vsb")
        nc.vector.memset(vsb[:], 1.0)
        for sb in range(SB):
            pv_ = psm.tile([P, P], f32, tag="small")
            for eb in range(EB):
                nc.tensor.matmul(
                    pv_[:], lhsT=xT[:, eb, sb * P:(sb + 1) * P], rhs=wv[:, eb, :],
                    start=(eb == 0), stop=False)
            nc.tensor.matmul(pv_[:], lhsT=ones1[:1, :], rhs=bv[:1, :],
                             start=False, stop=True)
            nc.vector.tensor_copy(vsb[:, sb, 0, 0:64], pv_[:, 0:64])
            nc.vector.tensor_copy(vsb[:, sb, 1, 0:64], pv_[:, 64:128])

        # ---- attention, one (batch, head) unit at a time ----
        attn = work.tile([P, SB, HPC * D], bf16, tag="attn")
        for b in range(B):
            for h in range(HPC):
                hs = slice(h * 64, (h + 1) * 64)
                expst = []
                for kb in range(SBB):
                    L = S - kb * P
                    et = epool.tile([P, L], bf16, tag=f"e{kb}")
                    off = b * S + kb * P
                    pos = 0
                    while pos < L:
                        c = min(512, L - pos)
                        ps = pbig.tile([P, 512], f32, tag="big")
                        nc.tensor.matmul(
                            ps[:, :c],
                            lhsT=qkT[hs, 1, off:off + P],
                            rhs=qkT[hs, 0, off + pos:off + pos + c],
                            start=True, stop=True)
                        nc.scalar.activation(
                            et[:, pos:pos + c], ps[:, :c],
                            mybir.ActivationFunctionType.Exp)
                        pos += c
                    # zero the invalid (q < k) half of the diagonal block
                    nc.vector.tensor_mul(et[:, 0:P], et[:, 0:P], tri[:])
                    expst.append(et)
                for qt in range(SBB):
                    pp = psm.tile([P, 66], f32, tag="small")
                    for kb in range(qt + 1):
                        nc.tensor.matmul(
                            pp[:, 0:65],
                            lhsT=expst[kb][:, (qt - kb) * P:(qt - kb) * P + P],
                            rhs=vsb[:, b * SBB + kb, h, 0:65],
                            start=(kb == 0), stop=(kb == qt))
                    rc = small.tile([P, 1], f32, tag="recip")
                    nc.vector.reciprocal(rc[:], pp[:, 64:65])
                    nc.vector.tensor_scalar_mul(
                        attn[:, b * SBB + qt, hs], pp[:, 0:64], rc[:])

        # ---- transpose attention output: [token, e] -> [e, token] ----
        attnT = work.tile([P, BS], bf16, tag="attnT")
        for blk in range(SB):
            pt = psm.tile([P, P], bf16, tag="small")
            nc.tensor.transpose(pt[:], attn[:, blk, :], ident[:])
            nc.vector.tensor_copy(attnT[:, blk * P:(blk + 1) * P], pt[:])

        # ---- AllToAll: head-columns -> token-rows ----
        for j in range(NCORES):
            nc.sync.dma_start(a2a_in[j], attnT[:, j * CH:(j + 1) * CH])
        nc.gpsimd.collective_compute(
            "AllToAll", mybir.AluOpType.bypass,
            replica_groups=[list(range(NCORES))],
            ins=[a2a_in.ap()], outs=[a2a_out.ap()])
        atf = work.tile([P, EB, CH], bf16, tag="atf")
        for j in range(NCORES):
            nc.sync.dma_start(atf[:, j, :], a2a_out[j])

        # ---- output projection for my 512 rows ----
        for st in range(CH // P):
            ot = opool.tile([P, E], f32, tag="o")
            for oh in range(2):
                po = pbig.tile([P, 512], f32, tag="big")
                for eb in range(EB):
                    nc.tensor.matmul(
                        po[:],
                        lhsT=atf[:, eb, st * P:(st + 1) * P],
                        rhs=wo[:, eb, oh * 512:(oh + 1) * 512],
                        start=(eb == 0), stop=False)
                nc.tensor.matmul(po[:], lhsT=ones1[:1, :],
                                 rhs=bo[:1, oh * 512:(oh + 1) * 512],
                                 start=False, stop=True)
                nc.vector.tensor_copy(ot[:, oh * 512:(oh + 1) * 512], po[:])
            nc.sync.dma_start(out_d[st * P:(st + 1) * P, :], ot[:])

    nc.compile()
    return nc


def _in_maps(x, W_qkv, b_qkv, W_o, b_o):
    xT = np.ascontiguousarray(
        x.reshape(BS, E).T).reshape(EB, P, BS).astype(_bf16)
    wo = np.ascontiguousarray(W_o.reshape(EB, P, E)).astype(_bf16)
    bo = np.asarray(b_o).reshape(1, E).astype(_bf16)
    tri = np.triu(np.ones((P, P), np.float32)).astype(_bf16)
    maps = []
    for c in range(NCORES):
        o = c * HPC * D
        q_sl = slice(o, o + HPC * D)
        k_sl = slice(E + o, E + o + HPC * D)
        v_sl = slice(2 * E + o, 2 * E + o + HPC * D)
        wqk = np.concatenate(
            [W_qkv[:, q_sl] * 0.125, W_qkv[:, k_sl]], axis=1)
        maps.append({
            "xT": xT,
            "wqk": np.ascontiguousarray(
                wqk.reshape(EB, P, 2 * P)).astype(_bf16),
            "wv": np.ascontiguousarray(
                W_qkv[:, v_sl].reshape(EB, P, P)).astype(_bf16),
            "wo": wo,
            "bqk": np.stack([b_qkv[q_sl] * 0.125,
                             b_qkv[k_sl]]).astype(np.float32),
            "bv": b_qkv[v_sl].reshape(1, P).astype(_bf16),
            "bo": bo,
            "tri": tri,
        })
    return maps


def kernel(x, W_qkv, b_qkv, W_o, b_o, mask):
    from concourse.bass_utils import run_bass_kernel_spmd

    if "nc" not in _cache:
        _cache["nc"] = _build()
    nc = _cache["nc"]
    maps = _in_maps(np.asarray(x, np.float32), np.asarray(W_qkv, np.float32),
                    np.asarray(b_qkv, np.float32), np.asarray(W_o, np.float32),
                    np.asarray(b_o, np.float32))
    res = run_bass_kernel_spmd(nc, maps, list(range(NCORES)))
    out = np.concatenate([res.results[c]["out"] for c in range(NCORES)], axis=0)
    return out.reshape(B, S, E).astype(np.float32)


# revision 42
# speedup vs baseline: 1.0315x; 1.0315x over previous
"""Multi-head causal self-attention (B=2, S=2048, E=1024, H=16) on 8 TRN2 NeuronCores.

Sharding: tensor-parallel over heads (2 heads/core, both batches). Per core:
  - QKV projection for its 2 heads (q^T/k^T transposed layout, v natural)
  - causal flash-style attention, scores computed transposed (k on partitions)
    so no probability-matrix transposes are needed; softmax denominator comes
    from a ones-column appended to V
  - PE-transpose of the attention output, 8-way AllToAll to reshard from
    head-columns to token-rows, then row-parallel output projection.
Host side only reshapes/slices inputs and concatenates the 8 disjoint row
shards of the output.

Emission order pipelines batch 1's QKV under batch 0's (ACT-bound) attention.
"""

import numpy as np
import ml_dtypes

P = 128
B, S, E, H, D = 2, 2048, 1024, 16, 64
NCORES = 8
EB = E // P            # 8 e-blocks
BS = B * S             # 4096 flattened rows
SBB = S // P           # 16 s-blocks per batch
SB = BS // P           # 32 s-blocks global
HPC = H // NCORES      # 2 heads per core
CH = BS // NCORES      # 512 rows owned per core

_bf16 = ml_dtypes.bfloat16
_cache = {}


def _build(no_cc=False):
    from contextlib import ExitStack

    import concourse.tile as tile
    from concourse import bacc, mybir
    from concourse.masks import make_identity

    bf16 = mybir.dt.bfloat16
    f32 = mybir.dt.float32

    nc = bacc.Bacc("TRN2", target_bir_lowering=False, debug=False,
                   num_devices=NCORES)

    # host-side layouts are [partition, eblock, col] so each load is one DMA
    xT_d = nc.dram_tensor("xT", [P, EB, BS], bf16, kind="ExternalInput")
    wqk_d = nc.dram_tensor("wqk", [P, EB, 2 * P], bf16, kind="ExternalInput")
    wv_d = nc.dram_tensor("wv", [P, EB, P], bf16, kind="ExternalInput")
    wo_d = nc.dram_tensor("wo", [P, EB, E], bf16, kind="ExternalInput")
    bqk_d = nc.dram_tensor("bqk", [P, 2], f32, kind="ExternalInput")
    bv_d = nc.dram_tensor("bv", [1, P], bf16, kind="ExternalInput")
    bo_d = nc.dram_tensor("bo", [1, E], bf16, kind="ExternalInput")
    tri_d = nc.dram_tensor("tri", [P, P], bf16, kind="ExternalInput")
    # rank r owns interleaved token blocks {r, r+8, r+16, r+24}: one AllToAll
    # per batch, so batch 0's A2A + output projection hide under batch 1's
    # attention. out row-block st <-> global block st*8 + rank.
    out_d = nc.dram_tensor("out", [4, P, E], f32, kind="ExternalOutput")
    a2a_in = [nc.dram_tensor(f"a2a_in{b}", [NCORES, P, 2 * P], bf16)
              for b in range(B)]
    a2a_out = [nc.dram_tensor(f"a2a_out{b}", [NCORES, P, 2 * P], bf16)
               for b in range(B)]

    with tile.TileContext(nc) as tc, ExitStack() as ctx:
        consts = ctx.enter_context(tc.tile_pool(name="consts", bufs=1))
        work = ctx.enter_context(tc.tile_pool(name="work", bufs=1))
        xpool = ctx.enter_context(tc.tile_pool(name="xstream", bufs=2))
        epool = ctx.enter_context(tc.tile_pool(name="expst", bufs=2))
        small = ctx.enter_context(tc.tile_pool(name="small", bufs=4))
        opool = ctx.enter_context(tc.tile_pool(name="osb", bufs=2))
        pbig = ctx.enter_context(tc.tile_pool(name="pbig", bufs=2, space="PSUM"))
        ppv = ctx.enter_context(tc.tile_pool(name="ppv", bufs=2, space="PSUM"))
        psm = ctx.enter_context(tc.tile_pool(name="psm", bufs=2, space="PSUM"))

        wqk = consts.tile([P, EB, 2 * P], bf16, tag="wqk")
        wv = consts.tile([P, EB, P], bf16, tag="wv")
        wo = consts.tile([P, EB, E], bf16, tag="wo")
        bqk = consts.tile([P, 2], f32, tag="bqk")
        bv = consts.tile([1, P], bf16, tag="bv")
        bo = consts.tile([1, E], bf16, tag="bo")
        tri = consts.tile([P, P], bf16, tag="tri")
        ones1 = consts.tile([1, P], bf16, tag="ones1")
        ident = consts.tile([P, P], bf16, tag="ident")

        nc.sync.dma_start(wqk[:], wqk_d[:, :, :])
        nc.sync.dma_start(wv[:], wv_d[:, :, :])
        nc.sync.dma_start(bqk[:], bqk_d[:, :])
        nc.sync.dma_start(bv[:1, :], bv_d[:, :])
        nc.sync.dma_start(tri[:], tri_d[:, :])
        nc.vector.memset(ones1[:1, :], 1.0)
        make_identity(nc, ident[:])

        qkT = [work.tile([P, 2, S], bf16, tag=f"qkT{b}", name=f"qkT{b}")
               for b in range(B)]
        vsb = [work.tile([P, SBB, HPC, 66], bf16, tag=f"vsb{b}", name=f"vsb{b}")
               for b in range(B)]
        attn = [work.tile([P, SBB, HPC * D], bf16, tag=f"attn{b}", name=f"attn{b}")
                for b in range(B)]
        attnT = [work.tile([P, S], bf16, tag=f"attnT{b}", name=f"attnT{b}")
                 for b in range(B)]

        def qkv_pieces(b):
            """QKV projection for batch b, one 512-token chunk per piece.

            Chunks are emitted suffix-first: causal score block kb only needs
            token columns >= kb*128, so late chunks unblock the small k-blocks
            early and ACT (exp) can start before the whole projection is done.
            """
            nc.vector.memset(vsb[b][:], 1.0)
            for sc in reversed(range(S // 512)):
                gc = b * S + sc * 512  # global col
                xc = xpool.tile([P, EB, 512], bf16, tag="xc", name="xc")
                nc.sync.dma_start(xc[:], xT_d[:, :, gc:gc + 512])
                for db in range(2):
                    ps = psm.tile([P, 512], f32, tag="mid", name="psqk")
                    for eb in range(EB):
                        nc.tensor.matmul(
                            ps[:],
                            lhsT=wqk[:, eb, db * P:(db + 1) * P],
                            rhs=xc[:, eb, :],
                            start=(eb == 0), stop=(eb == EB - 1),
                        )
                    nc.vector.tensor_scalar_add(
                        qkT[b][:, db, sc * 512:(sc + 1) * 512], ps[:],
                        bqk[:, db:db + 1])
                    yield
                for si in range(4):
                    sb = sc * 4 + si
                    pv_ = psm.tile([P, P], f32, tag="mid", name="psv")
                    for eb in range(EB):
                        nc.tensor.matmul(
                            pv_[:], lhsT=xc[:, eb, si * P:(si + 1) * P],
                            rhs=wv[:, eb, :], start=(eb == 0), stop=False)
                    nc.tensor.matmul(pv_[:], lhsT=ones1[:1, :], rhs=bv[:1, :],
                                     start=False, stop=True)
                    nc.vector.tensor_copy(vsb[b][:, sb, 0, 0:64], pv_[:, 0:64])
                    nc.vector.tensor_copy(vsb[b][:, sb, 1, 0:64], pv_[:, 64:128])
                    yield

        def score_pieces(b, h, expst):
            """scores^T + exp for one (batch, head), one k-block per piece.

            k-blocks run high-to-low to match qkv_pieces' suffix-first order.
            """
            hs = slice(h * 64, (h + 1) * 64)
            expst.extend([None] * SBB)
            for kb in reversed(range(SBB)):
                L = S - kb * P
                # high-kb tiles are small: an extra buffer lets the next
                # batch's (reversed) scores start while this batch's PV is
                # still reading the lower k-blocks
                et = epool.tile([P, L], bf16, tag=f"e{kb}", name=f"e{kb}",
                                bufs=4 if kb >= 8 else 2)
                off = kb * P
                pos = 0
                while pos < L:  # 1024-wide psum tiles: 1 exp op per tile
                    c = min(1024, L - pos)
                    ps = pbig.tile([P, 1024], f32, tag="big", name="pssc")
                    for c0 in range(0, c, 512):
                        w = min(512, c - c0)
                        nc.tensor.matmul(
                            ps[:, c0:c0 + w],
                            lhsT=qkT[b][hs, 1, off:off + P],
                            rhs=qkT[b][hs, 0, off + pos + c0:off + pos + c0 + w],
                            start=True, stop=True)
                    nc.scalar.activation(
                        et[:, pos:pos + c], ps[:, :c],
                        mybir.ActivationFunctionType.Exp)
                    pos += c
                # zero the invalid (q < k) half of the diagonal block
                nc.gpsimd.tensor_mul(et[:, 0:P], et[:, 0:P], tri[:])
                expst[kb] = et
                yield

        def pv_pieces(b, h, expst):
            """PV + normalize for one (batch, head), one q-tile per piece."""
            hs = slice(h * 64, (h + 1) * 64)
            for qt in range(SBB):
                pp = ppv.tile([P, 66], f32, tag="pv", name="pspv")
                for kb in range(qt + 1):
                    nc.tensor.matmul(
                        pp[:, 0:65],
                        lhsT=expst[kb][:, (qt - kb) * P:(qt - kb) * P + P],
                        rhs=vsb[b][:, kb, h, 0:65],
                        start=(kb == 0), stop=(kb == qt))
                rc = small.tile([P, 1], f32, tag="recip", name="rc")
                nc.vector.reciprocal(rc[:], pp[:, 64:65])
                nc.vector.tensor_scalar_mul(
                    attn[b][:, qt, hs], pp[:, 0:64], rc[:])
                yield

        def transpose_pieces(b):
            """attn [token, e] -> attnT [e, token] via PE transposes."""
            for blk in range(SBB):
                pt = psm.tile([P, P], bf16, tag="mid", name="pst")
                nc.tensor.transpose(pt[:], attn[b][:, blk, :], ident[:])
                nc.vector.tensor_copy(attnT[b][:, blk * P:(blk + 1) * P], pt[:])
                yield
            attnT_blocks = attnT[b][:, :].rearrange("p (t c) -> p t c", c=P)
            for j in range(NCORES):  # chunk j: rank j's blocks {j, j+8}
                nc.sync.dma_start(
                    a2a_in[b][j].rearrange("p (t c) -> p t c", t=2),
                    attnT_blocks[:, j::8, :])
            yield

        def interleave(*gens):
            gens = list(gens)
            while gens:
                gens = [g for g in gens if next(g, StopIteration) is not StopIteration]

        def paced(qg, score_gens, pv_gens=()):
            """Weave one qkv stream with score/pv streams, pacing emission so
            every score k-block is emitted AFTER the qkv chunk that writes the
            qkT columns it reads (Tile only tracks writer->reader deps in
            emission order). qkv chunk g (suffix-first) unlocks score k-blocks
            [12-4g, 15-4g]."""
            for g in range(4):
                for _ in range(6):
                    next(qg, None)
                for _ in range(4):
                    for sg in score_gens:
                        next(sg, None)
                    for pg in pv_gens:
                        next(pg, None)
            interleave(qg, *score_gens, *pv_gens)

        atf = [work.tile([P, EB, 2 * P], bf16, tag=f"atf{b}", name=f"atf{b}")
               for b in range(B)]

        def a2a_batch(b):
            """AllToAll batch b: head-columns -> my two token blocks."""
            if no_cc:
                # cost-model variant: TimelineSim can't simulate collectives
                for j in range(NCORES):
                    nc.sync.dma_start(a2a_out[b][j], a2a_in[b][j])
            else:
                nc.gpsimd.collective_compute(
                    "AllToAll", mybir.AluOpType.bypass,
                    replica_groups=[list(range(NCORES))],
                    ins=[a2a_in[b].ap()], outs=[a2a_out[b].ap()])
            for j in range(NCORES):
                nc.sync.dma_start(atf[b][:, j, :], a2a_out[b][j])

        def oproj_batch(b):
            """Output projection of my two token blocks of batch b."""
            for st in range(2):
                ot = opool.tile([P, E], f32, tag="o", name="ot")
                po = pbig.tile([P, 1024], f32, tag="big", name="pso")
                for oh in range(2):
                    for eb in range(EB):
                        nc.tensor.matmul(
                            po[:, oh * 512:(oh + 1) * 512],
                            lhsT=atf[b][:, eb, st * P:(st + 1) * P],
                            rhs=wo[:, eb, oh * 512:(oh + 1) * 512],
                            start=(eb == 0), stop=False)
                    nc.tensor.matmul(po[:, oh * 512:(oh + 1) * 512],
                                     lhsT=ones1[:1, :],
                                     rhs=bo[:1, oh * 512:(oh + 1) * 512],
                                     start=False, stop=True)
                nc.vector.tensor_copy(ot[:], po[:])
                nc.sync.dma_start(out_d[b * 2 + st], ot[:])

        # ---- pipelined emission (priorities; Tile schedules by readiness) ----
        e00, e01, e10, e11 = [], [], [], []
        paced(qkv_pieces(0),
              [score_pieces(0, 0, e00), score_pieces(0, 1, e01)])
        paced(qkv_pieces(1),
              [score_pieces(1, 0, e10), score_pieces(1, 1, e11)],
              [pv_pieces(0, 0, e00), pv_pieces(0, 1, e01)])
        nc.sync.dma_start(wo[:], wo_d[:, :, :])  # needed only at out-proj
        nc.sync.dma_start(bo[:1, :], bo_d[:, :])
        interleave(pv_pieces(1, 0, e10), pv_pieces(1, 1, e11),
                   transpose_pieces(0))
        a2a_batch(0)
        oproj_batch(0)          # hides under batch-1 attention tail
        interleave(transpose_pieces(1))
        a2a_batch(1)
        oproj_batch(1)

    nc.compile()
    return nc


def _in_maps(x, W_qkv, b_qkv, W_o, b_o):
    # [partition, eblock, col] layouts (see dram tensor decls)
    xT = np.ascontiguousarray(
        x.reshape(BS, EB, P).transpose(2, 1, 0)).astype(_bf16)
    wo = np.ascontiguousarray(
        W_o.reshape(EB, P, E).transpose(1, 0, 2)).astype(_bf16)
    bo = np.asarray(b_o).reshape(1, E).astype(_bf16)
    tri = np.triu(np.ones((P, P), np.float32)).astype(_bf16)
    maps = []
    for c in range(NCORES):
        o = c * HPC * D
        q_sl = slice(o, o + HPC * D)
        k_sl = slice(E + o, E + o + HPC * D)
        v_sl = slice(2 * E + o, 2 * E + o + HPC * D)
        wqk = np.concatenate(
            [W_qkv[:, q_sl] * 0.125, W_qkv[:, k_sl]], axis=1)
        maps.append({
            "xT": xT,
            "wqk": np.ascontiguousarray(
                wqk.reshape(EB, P, 2 * P).transpose(1, 0, 2)).astype(_bf16),
            "wv": np.ascontiguousarray(
                W_qkv[:, v_sl].reshape(EB, P, P).transpose(1, 0, 2)).astype(_bf16),
            "wo": wo,
            "bqk": np.stack([b_qkv[q_sl] * 0.125,
                             b_qkv[k_sl]], axis=1).astype(np.float32),
            "bv": b_qkv[v_sl].reshape(1, P).astype(_bf16),
            "bo": bo,
            "tri": tri,
        })
    return maps


def kernel(x, W_qkv, b_qkv, W_o, b_o, mask):
    from concourse.bass_utils import run_bass_kernel_spmd

    if "nc" not in _cache:
        _cache["nc"] = _build()
    nc = _cache["nc"]
    maps = _in_maps(np.asarray(x, np.float32), np.asarray(W_qkv, np.float32),
                    np.asarray(b_qkv, np.float32), np.asarray(W_o, np.float32),
                    np.asarray(b_o, np.float32))
    res = run_bass_kernel_spmd(nc, maps, list(range(NCORES)))
    # rank r's out[st] is global 128-token block st*8 + r
    full = np.empty((SB, P, E), np.float32)
    for r in range(NCORES):
        full[r::NCORES] = res.results[r]["out"]
    return full.reshape(B, S, E).astype(np.float32)


# revision 46
# speedup vs baseline: 168.8412x; 163.6774x over previous
"""Multi-head causal self-attention (B=2, S=2048, E=1024, H=16) on 8 TRN2 NeuronCores.

Sharding: tensor-parallel over heads (2 heads/core, both batches). Per core:
  - QKV projection for its 2 heads (q^T/k^T transposed layout, v natural)
  - causal flash-style attention, scores computed transposed (k on partitions)
    so no probability-matrix transposes are needed; softmax denominator comes
    from a ones-column appended to V
  - PE-transpose of the attention output, 8-way AllToAll to reshard from
    head-columns to token-rows, then row-parallel output projection.
Host side only reshapes/slices inputs and concatenates the 8 disjoint row
shards of the output.

Emission order pipelines batch 1's QKV under batch 0's (ACT-bound) attention.
"""

import numpy as np
import ml_dtypes

P = 128
B, S, E, H, D = 2, 2048, 1024, 16, 64
NCORES = 8
EB = E // P            # 8 e-blocks
BS = B * S             # 4096 flattened rows
SBB = S // P           # 16 s-blocks per batch
SB = BS // P           # 32 s-blocks global
HPC = H // NCORES      # 2 heads per core
CH = BS // NCORES      # 512 rows owned per core

_bf16 = ml_dtypes.bfloat16
_cache = {}


def _build(no_cc=False):
    from contextlib import ExitStack

    import concourse.tile as tile
    from concourse import bacc, mybir
    from concourse.masks import make_identity

    bf16 = mybir.dt.bfloat16
    f32 = mybir.dt.float32

    nc = bacc.Bacc("TRN2", target_bir_lowering=False, debug=False,
                   num_devices=NCORES)

    # host-side layouts are [partition, eblock, col] so each load is one DMA
    xT_d = nc.dram_tensor("xT", [P, EB, BS], bf16, kind="ExternalInput")
    wqk_d = nc.dram_tensor("wqk", [P, EB, 2 * P], bf16, kind="ExternalInput")
    wv_d = nc.dram_tensor("wv", [P, EB, P], bf16, kind="ExternalInput")
    wo_d = nc.dram_tensor("wo", [P, EB, E], bf16, kind="ExternalInput")
    bqk_d = nc.dram_tensor("bqk", [P, 2], f32, kind="ExternalInput")
    bv_d = nc.dram_tensor("bv", [1, P], bf16, kind="ExternalInput")
    bo_d = nc.dram_tensor("bo", [1, E], bf16, kind="ExternalInput")
    tri_d = nc.dram_tensor("tri", [P, P], bf16, kind="ExternalInput")
    # rank r owns interleaved token blocks {r, r+8, r+16, r+24}: one AllToAll
    # per batch, so batch 0's A2A + output projection hide under batch 1's
    # attention. out row-block st <-> global block st*8 + rank.
    out_d = nc.dram_tensor("out", [4, P, E], f32, kind="ExternalOutput")
    a2a_in = [nc.dram_tensor(f"a2a_in{b}", [NCORES, P, 2 * P], bf16)
              for b in range(B)]
    a2a_out = [nc.dram_tensor(f"a2a_out{b}", [NCORES, P, 2 * P], bf16)
               for b in range(B)]

    with tile.TileContext(nc) as tc, ExitStack() as ctx:
        consts = ctx.enter_context(tc.tile_pool(name="consts", bufs=1))
        work = ctx.enter_context(tc.tile_pool(name="work", bufs=1))
        xpool = ctx.enter_context(tc.tile_pool(name="xstream", bufs=2))
        epool = ctx.enter_context(tc.tile_pool(name="expst", bufs=2))
        small = ctx.enter_context(tc.tile_pool(name="small", bufs=4))
        opool = ctx.enter_context(tc.tile_pool(name="osb", bufs=2))
        pbig = ctx.enter_context(tc.tile_pool(name="pbig", bufs=2, space="PSUM"))
        ppv = ctx.enter_context(tc.tile_pool(name="ppv", bufs=2, space="PSUM"))
        psm = ctx.enter_context(tc.tile_pool(name="psm", bufs=2, space="PSUM"))

        wqk = consts.tile([P, EB, 2 * P], bf16, tag="wqk")
        wv = consts.tile([P, EB, P], bf16, tag="wv")
        wo = consts.tile([P, EB, E], bf16, tag="wo")
        bqk = consts.tile([P, 2], f32, tag="bqk")
        bv = consts.tile([1, P], bf16, tag="bv")
        bo = consts.tile([1, E], bf16, tag="bo")
        tri = consts.tile([P, P], bf16, tag="tri")
        ones1 = consts.tile([1, P], bf16, tag="ones1")
        ident = consts.tile([P, P], bf16, tag="ident")

        nc.sync.dma_start(wqk[:], wqk_d[:, :, :])
        nc.sync.dma_start(wv[:], wv_d[:, :, :])
        nc.sync.dma_start(bqk[:], bqk_d[:, :])
        nc.sync.dma_start(bv[:1, :], bv_d[:, :])
        nc.sync.dma_start(tri[:], tri_d[:, :])
        nc.vector.memset(ones1[:1, :], 1.0)
        make_identity(nc, ident[:])

        qkT = [work.tile([P, 2, S], bf16, tag=f"qkT{b}", name=f"qkT{b}")
               for b in range(B)]
        vsb = [work.tile([P, SBB, HPC, 66], bf16, tag=f"vsb{b}", name=f"vsb{b}")
               for b in range(B)]
        attn = [work.tile([P, SBB, HPC * D], bf16, tag=f"attn{b}", name=f"attn{b}")
                for b in range(B)]
        attnT = [work.tile([P, S], bf16, tag=f"attnT{b}", name=f"attnT{b}")
                 for b in range(B)]

        def qkv_pieces(b):
            """QKV projection for batch b, one 512-token chunk per piece.

            Chunks are emitted suffix-first: causal score block kb only needs
            token columns >= kb*128, so late chunks unblock the small k-blocks
            early and ACT (exp) can start before the whole projection is done.
            """
            nc.vector.memset(vsb[b][:], 1.0)
            for sc in reversed(range(S // 512)):
                gc = b * S + sc * 512  # global col
                xc = xpool.tile([P, EB, 512], bf16, tag="xc", name="xc")
                nc.sync.dma_start(xc[:], xT_d[:, :, gc:gc + 512])
                for db in range(2):
                    ps = psm.tile([P, 512], f32, tag="mid", name="psqk")
                    for eb in range(EB):
                        nc.tensor.matmul(
                            ps[:],
                            lhsT=wqk[:, eb, db * P:(db + 1) * P],
                            rhs=xc[:, eb, :],
                            start=(eb == 0), stop=(eb == EB - 1),
                        )
                    nc.vector.tensor_scalar_add(
                        qkT[b][:, db, sc * 512:(sc + 1) * 512], ps[:],
                        bqk[:, db:db + 1])
                    yield
                for si in range(4):
                    sb = sc * 4 + si
                    pv_ = psm.tile([P, P], f32, tag="mid", name="psv")
                    for eb in range(EB):
                        nc.tensor.matmul(
                            pv_[:], lhsT=xc[:, eb, si * P:(si + 1) * P],
                            rhs=wv[:, eb, :], start=(eb == 0), stop=False)
                    nc.tensor.matmul(pv_[:], lhsT=ones1[:1, :], rhs=bv[:1, :],
                                     start=False, stop=True)
                    nc.vector.tensor_copy(vsb[b][:, sb, 0, 0:64], pv_[:, 0:64])
                    nc.vector.tensor_copy(vsb[b][:, sb, 1, 0:64], pv_[:, 64:128])
                    yield

        def score_pieces(b, h, expst):
            """scores^T + exp for one (batch, head), one k-block per piece.

            k-blocks run high-to-low to match qkv_pieces' suffix-first order.
            """
            hs = slice(h * 64, (h + 1) * 64)
            expst.extend([None] * SBB)
            for kb in reversed(range(SBB)):
                L = S - kb * P
                # high-kb tiles are small: an extra buffer lets the next
                # batch's (reversed) scores start while this batch's PV is
                # still reading the lower k-blocks
                et = epool.tile([P, L], bf16, tag=f"e{kb}", name=f"e{kb}",
                                bufs=4 if kb >= 8 else 2)
                off = kb * P
                pos = 0
                while pos < L:  # 1024-wide psum tiles: 1 exp op per tile
                    c = min(1024, L - pos)
                    ps = pbig.tile([P, 1024], f32, tag="big", name="pssc")
                    for c0 in range(0, c, 512):
                        w = min(512, c - c0)
                        nc.tensor.matmul(
                            ps[:, c0:c0 + w],
                            lhsT=qkT[b][hs, 1, off:off + P],
                            rhs=qkT[b][hs, 0, off + pos + c0:off + pos + c0 + w],
                            start=True, stop=True)
                    nc.scalar.activation(
                        et[:, pos:pos + c], ps[:, :c],
                        mybir.ActivationFunctionType.Exp)
                    pos += c
                # zero the invalid (q < k) half of the diagonal block
                nc.gpsimd.tensor_mul(et[:, 0:P], et[:, 0:P], tri[:])
                expst[kb] = et
                yield

        def pv_pieces(b, h, expst):
            """PV + normalize for one (batch, head), one q-tile per piece."""
            hs = slice(h * 64, (h + 1) * 64)
            for qt in range(SBB):
                pp = ppv.tile([P, 66], f32, tag="pv", name="pspv")
                for kb in range(qt + 1):
                    nc.tensor.matmul(
                        pp[:, 0:65],
                        lhsT=expst[kb][:, (qt - kb) * P:(qt - kb) * P + P],
                        rhs=vsb[b][:, kb, h, 0:65],
                        start=(kb == 0), stop=(kb == qt))
                rc = small.tile([P, 1], f32, tag="recip", name="rc")
                nc.vector.reciprocal(rc[:], pp[:, 64:65])
                nc.vector.tensor_scalar_mul(
                    attn[b][:, qt, hs], pp[:, 0:64], rc[:])
                yield

        def transpose_pieces(b):
            """attn [token, e] -> attnT [e, token] via PE transposes."""
            for blk in range(SBB):
                pt = psm.tile([P, P], bf16, tag="mid", name="pst")
                nc.tensor.transpose(pt[:], attn[b][:, blk, :], ident[:])
                nc.vector.tensor_copy(attnT[b][:, blk * P:(blk + 1) * P], pt[:])
                yield
            # two strided DMAs: chunk j of the bounce gets blocks {j, j+8}
            for t in range(2):
                nc.sync.dma_start(
                    a2a_in[b].ap().rearrange(
                        "j p (t c) -> p j t c", t=2)[:, :, t, :],
                    attnT[b][:, t * NCORES * P:(t + 1) * NCORES * P].rearrange(
                        "p (j c) -> p j c", c=P))
            yield

        def interleave(*gens):
            gens = list(gens)
            while gens:
                gens = [g for g in gens if next(g, StopIteration) is not StopIteration]

        def paced(qg, score_gens, pv_gens=()):
            """Weave one qkv stream with score/pv streams, pacing emission so
            every score k-block is emitted AFTER the qkv chunk that writes the
            qkT columns it reads (Tile only tracks writer->reader deps in
            emission order). qkv chunk g (suffix-first) unlocks score k-blocks
            [12-4g, 15-4g]."""
            for g in range(4):
                for _ in range(6):
                    next(qg, None)
                for _ in range(4):
                    for sg in score_gens:
                        next(sg, None)
                    for pg in pv_gens:
                        next(pg, None)
            interleave(qg, *score_gens, *pv_gens)

        atf = [work.tile([P, EB, 2 * P], bf16, tag=f"atf{b}", name=f"atf{b}")
               for b in range(B)]

        def a2a_batch(b):
            """AllToAll batch b: head-columns -> my two token blocks."""
            if no_cc:
                # cost-model variant: TimelineSim can't simulate collectives
                for j in range(NCORES):
                    nc.sync.dma_start(a2a_out[b][j], a2a_in[b][j])
            else:
                nc.gpsimd.collective_compute(
                    "AllToAll", mybir.AluOpType.bypass,
                    replica_groups=[list(range(NCORES))],
                    ins=[a2a_in[b].ap()], outs=[a2a_out[b].ap()])
            nc.sync.dma_start(
                atf[b][:, :, :],
                a2a_out[b].ap().rearrange("j p c -> p j c"))

        def oproj_batch(b):
            """Output projection of my two token blocks of batch b."""
            for st in range(2):
                ot = opool.tile([P, E], f32, tag="o", name="ot")
                po = pbig.tile([P, 1024], f32, tag="big", name="pso")
                for oh in range(2):
                    for eb in range(EB):
                        nc.tensor.matmul(
                            po[:, oh * 512:(oh + 1) * 512],
                            lhsT=atf[b][:, eb, st * P:(st + 1) * P],
                            rhs=wo[:, eb, oh * 512:(oh + 1) * 512],
                            start=(eb == 0), stop=False)
                    nc.tensor.matmul(po[:, oh * 512:(oh + 1) * 512],
                                     lhsT=ones1[:1, :],
                                     rhs=bo[:1, oh * 512:(oh + 1) * 512],
                                     start=False, stop=True)
                nc.vector.tensor_copy(ot[:], po[:])
                nc.sync.dma_start(out_d[b * 2 + st], ot[:])

        # ---- pipelined emission (priorities; Tile schedules by readiness) ----
        e00, e01, e10, e11 = [], [], [], []
        paced(qkv_pieces(0),
              [score_pieces(0, 0, e00), score_pieces(0, 1, e01)])
        paced(qkv_pieces(1),
              [score_pieces(1, 0, e10), score_pieces(1, 1, e11)],
              [pv_pieces(0, 0, e00), pv_pieces(0, 1, e01)])
        nc.sync.dma_start(wo[:], wo_d[:, :, :])  # needed only at out-proj
        nc.sync.dma_start(bo[:1, :], bo_d[:, :])
        interleave(pv_pieces(1, 0, e10), pv_pieces(1, 1, e11),
                   transpose_pieces(0))
        a2a_batch(0)
        oproj_batch(0)          # hides under batch-1 attention tail
        interleave(transpose_pieces(1))
        a2a_batch(1)
        oproj_batch(1)

    nc.compile()
    return nc


def _in_maps(x, W_qkv, b_qkv, W_o, b_o):
    # [partition, eblock, col] layouts (see dram tensor decls)
    xT = np.ascontiguousarray(
        x.reshape(BS, EB, P).transpose(2, 1, 0)).astype(_bf16)
    wo = np.ascontiguousarray(
        W_o.reshape(EB, P, E).transpose(1, 0, 2)).astype(_bf16)
    bo = np.asarray(b_o).reshape(1, E).astype(_bf16)
    tri = np.triu(np.ones((P, P), np.float32)).astype(_bf16)
    maps = []
    for c in range(NCORES):
        o = c * HPC * D
        q_sl = slice(o, o + HPC * D)
        k_sl = slice(E + o, E + o + HPC * D)
        v_sl = slice(2 * E + o, 2 * E + o + HPC * D)
        wqk = np.concatenate(
            [W_qkv[:, q_sl] * 0.125, W_qkv[:, k_sl]], axis=1)
        maps.append({
            "xT": xT,
            "wqk": np.ascontiguousarray(
                wqk.reshape(EB, P, 2 * P).transpose(1, 0, 2)).astype(_bf16),
            "wv": np.ascontiguousarray(
                W_qkv[:, v_sl].reshape(EB, P, P).transpose(1, 0, 2)).astype(_bf16),
            "wo": wo,
            "bqk": np.stack([b_qkv[q_sl] * 0.125,
                             b_qkv[k_sl]], axis=1).astype(np.float32),
            "bv": b_qkv[v_sl].reshape(1, P).astype(_bf16),
            "bo": bo,
            "tri": tri,
        })
    return maps


def kernel(x, W_qkv, b_qkv, W_o, b_o, mask):
    from concourse.bass_utils import run_bass_kernel_spmd

    if "nc" not in _cache:
        _cache["nc"] = _build()
    nc = _cache["nc"]
    maps = _in_maps(np.asarray(x, np.float32), np.asarray(W_qkv, np.float32),
                    np.asarray(b_qkv, np.float32), np.asarray(W_o, np.float32),
                    np.asarray(b_o, np.float32))
    res = run_bass_kernel_spmd(nc, maps, list(range(NCORES)))
    # rank r's out[st] is global 128-token block st*8 + r
    full = np.empty((SB, P, E), np.float32)
    for r in range(NCORES):
        full[r::NCORES] = res.results[r]["out"]
    return full.reshape(B, S, E).astype(np.float32)
